# revision 8
# baseline (speedup 1.0000x reference)
"""Trainium2 Bass kernel for nn_NeuralODE_69166153334984.

Math notes (derived from the reference):
  - vector_field_2 params are all zero, so f1 = predictor(p1, p2, x) collapses
    to a CONSTANT field c = ln2 * rowsum(w1_3) + b1_3 (the JVP at zero params
    kills every x-dependent term).  ys1[b,k] = x0[b] + c * t_k, computed on
    device with a couple of instructions.
  - f2 = predictor(p2, p1, x) = -(sigmoid(z2) * (W2@(h2+d2)+b2)) @ w1_3^T with
    h_i the softplus forward chain and d_i the JVP chain; the forward head
    cancels.  The adaptive Tsit5 loop (I-controller, masked batch lanes) is
    replicated op-for-op in fp32.
  - FSAL: f(y) for the next step equals k7 (accepted) or the previous k1
    (rejected), bitwise, so each step needs only 6 MLP evals plus a select.

Layout: feature-major. Batch of 64 lanes per core on the free axis; MLP
features on partitions.  Linear layers as PE matmuls with stationary
transposed weights; z/U pairs fused via stacked [[W,W],[0,W]] weights; stage
combinations y + sum(A_ij K_j) via small PE matmuls against a K buffer.

Adaptive step count: static unroll of the measured per-interval max step
counts for the seed-0 data, then 2 If-guarded steps, then a For_i fallback up
to the reference's 40-step cap so arbitrary inputs still give exact results.
"""

import math

import numpy as np

import concourse.bass as bass
import concourse.bacc as bacc
import concourse.mybir as mybir
from concourse.tile import TileContext
from concourse.bass_utils import run_bass_kernel_spmd

dt = mybir.dt
OP = mybir.AluOpType
AF = mybir.ActivationFunctionType
AX = mybir.AxisListType

F32 = np.float32

# ---- solver config (matches reference) ----
RTOL, ATOL = 1e-3, 1e-6
SAFETY, FMIN, FMAX = 0.9, 0.2, 10.0
DT0 = 1e-3
MAX_STEPS = 40

A21 = 0.161
A31, A32 = -0.008480655492356989, 0.335480655492357
A41, A42, A43 = 2.8971530571054935, -6.359448489975075, 4.3622954328695815
A51, A52, A53, A54 = 5.325864828439257, -11.748883564062828, 7.4955393428898365, -0.09249506636175525
A61, A62, A63, A64, A65 = 5.86145544294642, -12.92096931784711, 8.159367898576159, -0.071584973281401, -0.028269050394068383
B1, B2, B3, B4, B5, B6 = 0.09646076681806523, 0.01, 0.4798896504144996, 1.379008574103742, -3.290069515436081, 2.324710524099774
E1 = -0.00178001105222577714
E2 = -0.0008164344596567469
E3 = 0.007880878010261995
E4 = -0.1447110071732629
E5 = 0.5823571654525552
E6 = -0.45808210592918697
E7 = 0.015151515151515152

N_CORES = 8
B_FULL = 512
N = B_FULL // N_CORES  # 64 lanes per core
T = 10
H = 32  # hidden width

# max steps actually needed per interval on the seed-0 data (measured on a
# bit-faithful host prototype); the If/For_i tail covers anything beyond.
STATIC_STEPS = [5, 2, 1, 1, 1, 1, 1, 1, 1]
N_IF_STEPS = 2

CLAMP = -60.0  # exp input clamp; softplus/sigmoid exact to ~1e-5 abs beyond


def _build_coefs() -> np.ndarray:
    """[2, 60] tile of 2x2 diagonal coefficient blocks for the accumulating
    stage matmuls: I2, A21, A31, A32, A41..A43, A51..A54, A61..A65,
    B1..B6, E1..E6, E7, ones22 (30 blocks, 2 cols each)."""
    blocks = ([1.0, A21, A31, A32, A41, A42, A43, A51, A52, A53, A54,
               A61, A62, A63, A64, A65, B1, B2, B3, B4, B5, B6,
               E1, E2, E3, E4, E5, E6, E7])
    c = np.zeros((2, 60), F32)
    for i, v in enumerate(blocks):
        c[0, 2 * i], c[1, 2 * i + 1] = v, v
    c[0:2, 58:60] = 1.0  # ones22
    return c


CO_I2 = 0
CO_A = [None, None, 1, 2, 4, 7, 11]  # first block idx of A_j* for stage j (j=2..6)
CO_B = 16
CO_E = 22
CO_E7 = 28
CO_ONES = 29


def build_bass(static_steps=STATIC_STEPS, n_if_steps=N_IF_STEPS, max_steps=MAX_STEPS,
               n_intervals=T - 1):
    nc = bacc.Bacc(target_bir_lowering=False)

    # ---- DRAM I/O (per core) ----
    x0_d = nc.dram_tensor("x0", [2, N], dt.float32, kind="ExternalInput")
    tb_d = nc.dram_tensor("tb", [2, T], dt.float32, kind="ExternalInput")
    w0t_d = nc.dram_tensor("w0t", [2, H], dt.float32, kind="ExternalInput")
    w1x_d = nc.dram_tensor("w1x", [2 * H, 2 * H], dt.float32, kind="ExternalInput")
    w2x_d = nc.dram_tensor("w2x", [2 * H, 2 * H], dt.float32, kind="ExternalInput")
    nw3t_d = nc.dram_tensor("nw3t", [H, 2], dt.float32, kind="ExternalInput")
    b0_d = nc.dram_tensor("b0", [H, 1], dt.float32, kind="ExternalInput")
    b1_d = nc.dram_tensor("b1", [H, 1], dt.float32, kind="ExternalInput")
    b2_d = nc.dram_tensor("b2", [H, 1], dt.float32, kind="ExternalInput")
    b13_d = nc.dram_tensor("b13", [2, 1], dt.float32, kind="ExternalInput")
    coefs_d = nc.dram_tensor("coefs", [2, 60], dt.float32, kind="ExternalInput")
    out1_d = nc.dram_tensor("out1", [2, T * N], dt.float32, kind="ExternalOutput")
    out2_d = nc.dram_tensor("out2", [2, T * N], dt.float32, kind="ExternalOutput")

    with TileContext(nc) as tc:
        with (
            tc.tile_pool(name="const", bufs=1) as cpool,
            tc.tile_pool(name="state", bufs=1) as spool,
            tc.tile_pool(name="scratch", bufs=2) as wpool,
            tc.tile_pool(name="psum", bufs=1, space="PSUM") as ppool,
        ):
            # ---- constants into SBUF ----
            x0 = cpool.tile([2, N], dt.float32, tag="x0")
            tb = cpool.tile([2, T], dt.float32, tag="tb")
            w0t = cpool.tile([2, H], dt.float32, tag="w0t")
            w1x = cpool.tile([2 * H, 2 * H], dt.float32, tag="w1x")
            w2x = cpool.tile([2 * H, 2 * H], dt.float32, tag="w2x")
            nw3t = cpool.tile([H, 2], dt.float32, tag="nw3t")
            b0 = cpool.tile([H, 1], dt.float32, tag="b0")
            b1 = cpool.tile([H, 1], dt.float32, tag="b1")
            b2 = cpool.tile([H, 1], dt.float32, tag="b2")
            b13 = cpool.tile([2, 1], dt.float32, tag="b13")
            coefs = cpool.tile([2, 60], dt.float32, tag="coefs")
            ones32 = cpool.tile([H, 1], dt.float32, tag="ones32")
            ones2N = cpool.tile([2, N], dt.float32, tag="ones2N")
            ln09t = cpool.tile([2, 1], dt.float32, tag="ln09t")
            for tile, dram in [(x0, x0_d), (tb, tb_d), (w0t, w0t_d), (w1x, w1x_d),
                               (w2x, w2x_d), (nw3t, nw3t_d), (b0, b0_d), (b1, b1_d),
                               (b2, b2_d), (b13, b13_d), (coefs, coefs_d)]:
                nc.sync.dma_start(tile[:], dram[:])
            nc.gpsimd.memset(ones32[:], 1.0)
            nc.gpsimd.memset(ln09t[:], float(np.log(SAFETY)))
            nc.gpsimd.memset(ones2N[:], 1.0)

            # ---- persistent state ----
            y = spool.tile([2, N], dt.float32, tag="y")
            K = [spool.tile([2, N], dt.float32, tag=f"K{j}", name=f"K{j}") for j in range(1, 8)]
            tt = spool.tile([2, N], dt.float32, tag="tt")
            dtt = spool.tile([2, N], dt.float32, tag="dtt")
            nd = spool.tile([2, N], dt.float32, tag="nd")
            rem = spool.tile([2, N], dt.float32, tag="rem")
            hh = spool.tile([2, N], dt.float32, tag="hh")
            k1raw = spool.tile([2, N], dt.float32, tag="k1raw")
            ys2acc = spool.tile([2, T * N], dt.float32, tag="ys2acc")
            out1acc = spool.tile([2, T * N], dt.float32, tag="out1acc")

            # ---- ys1: constant-field closed form ----
            psc = ppool.tile([2, 1], dt.float32, tag="psX")
            nc.tensor.matmul(psc[:], nw3t[:], ones32[:])  # [2,1] = -rowsum(w1_3)
            cconst = cpool.tile([2, 1], dt.float32, tag="cconst")
            # c = ln2 * rowsum + b13  (psc = -rowsum, so scale by -ln2)
            nc.scalar.activation(cconst[:], psc[:], AF.Identity,
                                 bias=b13[:, 0:1], scale=-float(np.log(2.0)))
            ct = cpool.tile([2, T], dt.float32, tag="ct")
            nc.vector.tensor_scalar(out=ct[:], in0=tb[:], scalar1=cconst[:, 0:1],
                                    scalar2=None, op0=OP.mult)
            for k in range(T):
                nc.vector.tensor_scalar(out=out1acc[:, k * N:(k + 1) * N], in0=x0[:],
                                        scalar1=ct[:, k:k + 1], scalar2=None, op0=OP.add)
            nc.sync.dma_start(out1_d[:], out1acc[:])

            # ---- init ODE state ----
            for Kj in K:
                nc.gpsimd.memset(Kj[:], 0.0)
            nc.vector.tensor_copy(y[:], x0[:])
            nc.vector.tensor_copy(ys2acc[:, 0:N], x0[:])
            nc.gpsimd.memset(dtt[:], DT0)

            def emit_eval(X):
                """f2 MLP+JVP eval; X is a [2, N] SBUF AP. Returns psf [2,N] PSUM."""
                ps0 = ppool.tile([H, N], dt.float32, tag="ps0")
                nc.tensor.matmul(ps0[:], w0t[:], X)
                z0p = wpool.tile([H, N], dt.float32, tag="z0p")
                nc.vector.tensor_scalar(out=z0p[:], in0=ps0[:], scalar1=b0[:, 0:1],
                                        scalar2=None, op0=OP.add)
                zc0 = wpool.tile([H, N], dt.float32, tag="zc0")
                nc.gpsimd.tensor_scalar(out=zc0[:], in0=z0p[:], scalar1=CLAMP,
                                        scalar2=None, op0=OP.max)
                e0 = wpool.tile([H, N], dt.float32, tag="e0")
                nc.scalar.activation(e0[:], zc0[:], AF.Exp, scale=-1.0)
                t0 = wpool.tile([H, N], dt.float32, tag="t0")
                nc.gpsimd.tensor_scalar(out=t0[:], in0=e0[:], scalar1=1.0,
                                        scalar2=None, op0=OP.add)
                l0 = wpool.tile([H, N], dt.float32, tag="l0")
                nc.scalar.activation(l0[:], t0[:], AF.Ln)
                hd1 = wpool.tile([2 * H, N], dt.float32, tag="hd1")
                nc.vector.tensor_tensor(out=hd1[0:H, :], in0=zc0[:], in1=l0[:], op=OP.add)
                q0 = wpool.tile([H, N], dt.float32, tag="q0")
                nc.scalar.activation(q0[:], l0[:], AF.Exp, scale=-1.0)
                nc.vector.tensor_tensor(out=hd1[H:2 * H, :], in0=q0[:], in1=z0p[:], op=OP.mult)

                psL1 = ppool.tile([2 * H, N], dt.float32, tag="psL1")
                nc.tensor.matmul(psL1[:], w1x[:], hd1[:])
                z1p = wpool.tile([H, N], dt.float32, tag="z1p")
                nc.vector.tensor_scalar(out=z1p[:], in0=psL1[0:H, :], scalar1=b1[:, 0:1],
                                        scalar2=None, op0=OP.add)
                zc1 = wpool.tile([H, N], dt.float32, tag="zc1")
                nc.gpsimd.tensor_scalar(out=zc1[:], in0=z1p[:], scalar1=CLAMP,
                                        scalar2=None, op0=OP.max)
                e1 = wpool.tile([H, N], dt.float32, tag="e1")
                nc.scalar.activation(e1[:], zc1[:], AF.Exp, scale=-1.0)
                t1 = wpool.tile([H, N], dt.float32, tag="t1")
                nc.gpsimd.tensor_scalar(out=t1[:], in0=e1[:], scalar1=1.0,
                                        scalar2=None, op0=OP.add)
                l1 = wpool.tile([H, N], dt.float32, tag="l1")
                nc.scalar.activation(l1[:], t1[:], AF.Ln)
                hd2 = wpool.tile([2 * H, N], dt.float32, tag="hd2")
                nc.vector.tensor_tensor(out=hd2[0:H, :], in0=zc1[:], in1=l1[:], op=OP.add)
                q1 = wpool.tile([H, N], dt.float32, tag="q1")
                nc.scalar.activation(q1[:], l1[:], AF.Exp, scale=-1.0)
                # d2 = (U1 + b1) * sigmoid(z1)
                nc.vector.scalar_tensor_tensor(out=hd2[H:2 * H, :], in0=psL1[H:2 * H, :],
                                               scalar=b1[:, 0:1], in1=q1[:],
                                               op0=OP.add, op1=OP.mult)

                psL2 = ppool.tile([2 * H, N], dt.float32, tag="psL2")
                nc.tensor.matmul(psL2[:], w2x[:], hd2[:])
                z2p = wpool.tile([H, N], dt.float32, tag="z2p")
                nc.vector.tensor_scalar(out=z2p[:], in0=psL2[0:H, :], scalar1=b2[:, 0:1],
                                        scalar2=None, op0=OP.add)
                zc2 = wpool.tile([H, N], dt.float32, tag="zc2")
                nc.gpsimd.tensor_scalar(out=zc2[:], in0=z2p[:], scalar1=CLAMP,
                                        scalar2=None, op0=OP.max)
                e2 = wpool.tile([H, N], dt.float32, tag="e2")
                nc.scalar.activation(e2[:], zc2[:], AF.Exp, scale=-1.0)
                t2 = wpool.tile([H, N], dt.float32, tag="t2")
                nc.gpsimd.tensor_scalar(out=t2[:], in0=e2[:], scalar1=1.0,
                                        scalar2=None, op0=OP.add)
                q2 = wpool.tile([H, N], dt.float32, tag="q2")
                q2s = wpool.tile([H, N], dt.float32, tag="q2s")
                nc.vector.reciprocal_approx_accurate(out=q2[:], in_=t2[:], scratch=q2s[:])
                d3 = wpool.tile([H, N], dt.float32, tag="d3")
                nc.vector.scalar_tensor_tensor(out=d3[:], in0=psL2[H:2 * H, :],
                                               scalar=b2[:, 0:1], in1=q2[:],
                                               op0=OP.add, op1=OP.mult)
                psf = ppool.tile([2, N], dt.float32, tag="psf")
                nc.tensor.matmul(psf[:], nw3t[:], d3[:])
                return psf

            def emit_combo(ps, terms):
                """ps += sum over (block_idx, rhs_tile): accumulating 2x2-block MMs."""
                for i, (bi, rhs) in enumerate(terms):
                    nc.tensor.matmul(ps[:], coefs[0:2, 2 * bi:2 * bi + 2], rhs,
                                     start=(i == 0), stop=(i == len(terms) - 1))

            # initial k1raw = f2(y0)
            psf0 = emit_eval(x0[:])
            nc.scalar.copy(k1raw[:], psf0[:])

            def emit_step(t1ap):
                # h = min(dt, rem); K1 = h * k1raw
                nc.vector.tensor_tensor(out=hh[:], in0=dtt[:], in1=rem[:], op=OP.min)
                nc.vector.tensor_tensor(out=K[0][:], in0=hh[:], in1=k1raw[:], op=OP.mult)
                # stages k2..k6
                for j in range(2, 7):
                    psX = ppool.tile([2, N], dt.float32, tag="psX")
                    emit_combo(psX, [(CO_I2, y[:])] + [
                        (CO_A[j] + i, K[i][:]) for i in range(j - 1)])
                    Xs = wpool.tile([2, N], dt.float32, tag="Xs")
                    nc.scalar.copy(Xs[:], psX[:])
                    psf = emit_eval(Xs[:])
                    nc.vector.tensor_tensor(out=K[j - 1][:], in0=psf[:],
                                            in1=hh[:], op=OP.mult)
                # y_new
                psyn = ppool.tile([2, N], dt.float32, tag="psyn")
                emit_combo(psyn, [(CO_I2, y[:])] + [
                    (CO_B + i, K[i][:]) for i in range(6)])
                yn = wpool.tile([2, N], dt.float32, tag="yn")
                nc.scalar.copy(yn[:], psyn[:])
                # k7 (FSAL)
                psf7 = emit_eval(yn[:])
                nc.vector.tensor_tensor(out=K[6][:], in0=psf7[:], in1=hh[:], op=OP.mult)
                k7sb = wpool.tile([2, N], dt.float32, tag="k7sb")
                nc.scalar.copy(k7sb[:], psf7[:])
                # err = sum E_j K_j
                pserr = ppool.tile([2, N], dt.float32, tag="pserr")
                emit_combo(pserr, [(CO_E + i, K[i][:]) for i in range(6)] + [(CO_E7, K[6][:])])
                # scale = ATOL + RTOL * max(|y|, |yn|); rs = 1/scale
                ay = wpool.tile([2, N], dt.float32, tag="ay")
                nc.scalar.activation(ay[:], y[:], AF.Abs)
                ayn = wpool.tile([2, N], dt.float32, tag="ayn")
                nc.scalar.activation(ayn[:], yn[:], AF.Abs)
                mx = wpool.tile([2, N], dt.float32, tag="mx")
                nc.vector.tensor_tensor(out=mx[:], in0=ay[:], in1=ayn[:], op=OP.max)
                sc = wpool.tile([2, N], dt.float32, tag="sc")
                nc.vector.tensor_scalar(out=sc[:], in0=mx[:], scalar1=float(RTOL),
                                        scalar2=float(ATOL), op0=OP.mult, op1=OP.add)
                rs = wpool.tile([2, N], dt.float32, tag="rs")
                rss = wpool.tile([2, N], dt.float32, tag="rss")
                nc.vector.reciprocal_approx_accurate(out=rs[:], in_=sc[:], scratch=rss[:])
                es = wpool.tile([2, N], dt.float32, tag="es")
                nc.vector.tensor_tensor(out=es[:], in0=pserr[:], in1=rs[:], op=OP.mult)
                es2 = wpool.tile([2, N], dt.float32, tag="es2")
                nc.vector.tensor_tensor(out=es2[:], in0=es[:], in1=es[:], op=OP.mult)
                psm = ppool.tile([2, N], dt.float32, tag="psm")
                nc.tensor.matmul(psm[:], coefs[0:2, 58:60], es2[:])
                # accept: (e0^2+e1^2) <= 2  <=>  mean <= 1
                acc = wpool.tile([2, N], dt.float32, tag="acc")
                nc.vector.tensor_scalar(out=acc[:], in0=psm[:], scalar1=2.0,
                                        scalar2=None, op0=OP.is_le)
                # factor = clip(0.9 * m^-0.1, 0.2, 10); m = 0.5*psm
                Lt = wpool.tile([2, N], dt.float32, tag="Lt")
                nc.scalar.activation(Lt[:], psm[:], AF.Ln, scale=0.5)
                fac = wpool.tile([2, N], dt.float32, tag="fac")
                nc.scalar.activation(fac[:], Lt[:], AF.Exp, bias=ln09t[:, 0:1], scale=-0.1)
                facc = wpool.tile([2, N], dt.float32, tag="facc")
                nc.vector.tensor_scalar(out=facc[:], in0=fac[:], scalar1=float(FMAX),
                                        scalar2=float(FMIN), op0=OP.min, op1=OP.max)
                # step_ok
                s = wpool.tile([2, N], dt.float32, tag="s")
                nc.vector.tensor_tensor(out=s[:], in0=acc[:], in1=nd[:], op=OP.mult)
                # t, y, k1 updates (masked)
                tn = wpool.tile([2, N], dt.float32, tag="tn")
                nc.vector.tensor_tensor(out=tn[:], in0=tt[:], in1=hh[:], op=OP.add)
                nc.vector.copy_predicated(out=tt[:], mask=s[:].bitcast(dt.int32), data=tn[:])
                nc.vector.copy_predicated(out=y[:], mask=s[:].bitcast(dt.int32), data=yn[:])
                nc.vector.copy_predicated(out=k1raw[:], mask=s[:].bitcast(dt.int32), data=k7sb[:])
                # dt update: where(notdone, max(h*facc, 1e-10), dt)
                hf = wpool.tile([2, N], dt.float32, tag="hf")
                nc.vector.tensor_tensor(out=hf[:], in0=hh[:], in1=facc[:], op=OP.mult)
                hfm = wpool.tile([2, N], dt.float32, tag="hfm")
                nc.gpsimd.tensor_scalar(out=hfm[:], in0=hf[:], scalar1=1e-10,
                                        scalar2=None, op0=OP.max)
                nc.vector.copy_predicated(out=dtt[:], mask=nd[:].bitcast(dt.int32), data=hfm[:])
                # rem = t1 - t ; done update ; notdone &= (rem > 1e-9)
                nc.vector.tensor_scalar(out=rem[:], in0=tt[:], scalar1=-1.0,
                                        scalar2=t1ap, op0=OP.mult, op1=OP.add)
                cmp = wpool.tile([2, N], dt.float32, tag="cmp")
                nc.gpsimd.tensor_scalar(out=cmp[:], in0=rem[:], scalar1=1e-9,
                                        scalar2=None, op0=OP.is_gt)
                nc.vector.tensor_tensor(out=nd[:], in0=nd[:], in1=cmp[:], op=OP.mult)

            # ---- interval loop ----
            for iv in range(n_intervals):
                t0ap = tb[:, iv:iv + 1]
                t1ap = tb[:, iv + 1:iv + 2]
                nc.vector.tensor_scalar(out=tt[:], in0=ones2N[:], scalar1=t0ap,
                                        scalar2=None, op0=OP.mult)
                nc.gpsimd.memset(nd[:], 1.0)
                nc.vector.tensor_scalar(out=rem[:], in0=tt[:], scalar1=-1.0,
                                        scalar2=t1ap, op0=OP.mult, op1=OP.add)
                for _ in range(static_steps[iv]):
                    emit_step(t1ap)
                if n_if_steps or max_steps > static_steps[iv]:
                    ndr = wpool.tile([1, 1], dt.float32, tag="ndr")
                    nc.vector.reduce_max(ndr[:], nd[0:1, :], axis=AX.X)
                    for _r in range(n_if_steps):
                        ndi = wpool.tile([1, 1], dt.int32, tag="ndi")
                        nc.vector.tensor_copy(ndi[:], ndr[:])
                        fv = nc.values_load(ndi[0:1, 0:1], min_val=0, max_val=1,
                                            skip_runtime_bounds_check=True)
                        with tc.If(fv > 0):
                            emit_step(t1ap)
                            nc.vector.reduce_max(ndr[:], nd[0:1, :], axis=AX.X)
                    tailn = max_steps - static_steps[iv] - n_if_steps
                    if tailn > 0:
                        ndt = wpool.tile([1, 1], dt.float32, tag="ndt")
                        nc.vector.tensor_scalar(out=ndt[:], in0=ndr[:],
                                                scalar1=float(tailn), scalar2=None,
                                                op0=OP.mult)
                        ndti = wpool.tile([1, 1], dt.int32, tag="ndti")
                        nc.vector.tensor_copy(ndti[:], ndt[:])
                        fv2 = nc.values_load(ndti[0:1, 0:1], min_val=0, max_val=tailn,
                                             skip_runtime_bounds_check=True)
                        with tc.For_i(0, fv2) as _i:
                            emit_step(t1ap)
                nc.vector.tensor_copy(ys2acc[:, (iv + 1) * N:(iv + 2) * N], y[:])

            nc.sync.dma_start(out2_d[:], ys2acc[:])

    nc.compile()
    return nc


_CACHE: dict = {}


def _get_nc():
    if "nc" not in _CACHE:
        _CACHE["nc"] = build_bass()
    return _CACHE["nc"]


def _prepare_in_maps(inputs):
    x0s = np.asarray(inputs["x0s"], F32)          # [512, 2]
    t_eval = np.asarray(inputs["t_eval"], F32)    # [10]
    w1_0 = np.asarray(inputs["w1_0"], F32)
    b1_0 = np.asarray(inputs["b1_0"], F32)
    w1_1 = np.asarray(inputs["w1_1"], F32)
    b1_1 = np.asarray(inputs["b1_1"], F32)
    w1_2 = np.asarray(inputs["w1_2"], F32)
    b1_2 = np.asarray(inputs["b1_2"], F32)
    w1_3 = np.asarray(inputs["w1_3"], F32)
    b1_3 = np.asarray(inputs["b1_3"], F32)

    W0T = np.ascontiguousarray(w1_0.T)            # [2, 32]
    W1T = w1_1.T
    W2T = w1_2.T
    w1x = np.zeros((2 * H, 2 * H), F32)
    w1x[0:H, 0:H] = W1T
    w1x[0:H, H:2 * H] = W1T
    w1x[H:2 * H, H:2 * H] = W1T
    w2x = np.zeros((2 * H, 2 * H), F32)
    w2x[0:H, 0:H] = W2T
    w2x[0:H, H:2 * H] = W2T
    w2x[H:2 * H, H:2 * H] = W2T
    nw3t = np.ascontiguousarray(-w1_3.T)          # [32, 2]
    tbv = np.broadcast_to(t_eval[None, :], (2, T)).copy()
    coefs = _build_coefs()

    shared = {
        "tb": tbv, "w0t": W0T, "w1x": w1x, "w2x": w2x, "nw3t": nw3t,
        "b0": b1_0.reshape(H, 1).copy(), "b1": b1_1.reshape(H, 1).copy(),
        "b2": b1_2.reshape(H, 1).copy(), "b13": b1_3.reshape(2, 1).copy(),
        "coefs": coefs,
    }
    x0T = np.ascontiguousarray(x0s.T)             # [2, 512]
    in_maps = []
    for i in range(N_CORES):
        m = dict(shared)
        m["x0"] = np.ascontiguousarray(x0T[:, i * N:(i + 1) * N])
        in_maps.append(m)
    return in_maps


def _run(inputs, trace=False):
    nc = _get_nc()
    in_maps = _prepare_in_maps(inputs)
    res = run_bass_kernel_spmd(nc, in_maps, core_ids=list(range(N_CORES)), trace=trace)
    ys1_parts, ys2_parts = [], []
    for i in range(N_CORES):
        o1 = np.asarray(res.results[i]["out1"]).reshape(2, T, N).transpose(2, 1, 0)
        o2 = np.asarray(res.results[i]["out2"]).reshape(2, T, N).transpose(2, 1, 0)
        ys1_parts.append(o1)
        ys2_parts.append(o2)
    ys1 = np.concatenate(ys1_parts, axis=0).astype(F32)
    ys2 = np.concatenate(ys2_parts, axis=0).astype(F32)
    return (ys1, ys2), res


def kernel(**inputs):
    out, _ = _run(inputs, trace=False)
    return out
